# revision 1
# baseline (speedup 1.0000x reference)
"""ColBERT MaxSim scoring kernel for Trainium2 (8 NeuronCores, data-parallel over batch).

Strategy (per core, 128 samples):
  Host prep (index-only): dedup the core's token ids; build int16 gather index
  tables; fold doc-attention-mask into the token->slot mapping (masked doc
  tokens point at a zeroed table slot); rearrange query mask for the epilogue.

  Device:
   1. dma_gather the unique embedding rows (f32, 3KB each) from embed_table.
   2. Per 128-row tile: cast fp16 -> PE transpose -> project with proj_w
      (fp16 matmuls, f32 accum) + bias -> L2-normalize -> store into an
      SBUF-resident local table LT (row-major fp16, 256B rows).
   3. SBUF-source transposed dma_gather pulls per-token embeddings out of LT
      directly in [emb_dim x tokens] layout (contraction dim on partitions).
   4. Per sample: one 32x180 fp16 matmul (4 samples packed in the PE array via
      column tiling) -> row-max on DVE -> weighted sum against the query mask
      via a small f32r matmul -> scores.
"""
import sys
import numpy as np

sys.path.insert(0, "/opt/trn_rl_repo")

VOCAB, HIDDEN, EMB = 30522, 768, 128
B, QLEN, DLEN = 1024, 32, 180
N_CORES = 8
BS = B // N_CORES          # 128 samples per core
NQ = BS * QLEN             # 4096 query tokens per core
ND = BS * DLEN             # 23040 doc tokens per core
KCH = HIDDEN // 128        # 6 contraction chunks
CH = 8                     # vocab tiles gathered per dma_gather

_PROG_CACHE = {}


def _wrap_idx(ids):
    """dma_gather index layout: idx i -> [i % 16, i // 16], replicated to 128 partitions."""
    ids = np.asarray(ids)
    n = ids.size
    ncol = -(-n // 16)
    flat = np.zeros(ncol * 16, np.int16)
    flat[:n] = ids.astype(np.int16)
    a = flat.reshape(ncol, 16).T
    return np.ascontiguousarray(np.tile(a, (8, 1)))


def _prep_core(qi, qm, di, dm):
    """Host-side index prep for one core's 128 samples."""
    all_ids = np.concatenate([qi.reshape(-1), di.reshape(-1)]).astype(np.int64)
    uniq, inv = np.unique(all_ids, return_inverse=True)
    slot_q = inv[:NQ].astype(np.int32)
    slot_d = inv[NQ:].astype(np.int32)
    return uniq, slot_q, slot_d


def _build_program(u_fix, repeat=None, internal_table=False):
    """repeat/internal_table are used only by the timing harness (test.py):
    the whole pipeline is wrapped in a hardware loop and the big table input
    is replaced by an internal (non-uploaded) DRAM tensor."""
    import contextlib
    import concourse.bacc as bacc
    import concourse.mybir as mybir
    from concourse.tile import TileContext
    from concourse.masks import make_identity

    f32, f16, i16 = mybir.dt.float32, mybir.dt.float16, mybir.dt.int16
    f32r = mybir.dt.float32r
    P = 128
    n_tiles = u_fix // 128
    n_stripes = n_tiles + 1            # +1 zero stripe for masked doc tokens

    nc = bacc.Bacc("TRN2", target_bir_lowering=False, debug=False, num_devices=1,
                   dynamic_dma_scratch_size=65536, num_swdge_queues=2)

    if internal_table:
        table = nc.dram_tensor("table_int", [VOCAB, HIDDEN], f32)
    else:
        table = nc.dram_tensor("table", [VOCAB, HIDDEN], f32, kind="ExternalInput")
    w_in = nc.dram_tensor("w", [HIDDEN, EMB], f32, kind="ExternalInput")
    b_in = nc.dram_tensor("b", [1, EMB], f32, kind="ExternalInput")
    g1_in = nc.dram_tensor("g1_idx", [P, u_fix // 16], i16, kind="ExternalInput")
    gq_in = nc.dram_tensor("gq_idx", [P, NQ // 16], i16, kind="ExternalInput")
    gd_in = nc.dram_tensor("gd_idx", [P, ND // 16], i16, kind="ExternalInput")
    qm_in = nc.dram_tensor("qmask_r", [P, BS // 4], f32, kind="ExternalInput")
    g_in = nc.dram_tensor("gmat", [P, 4], f32, kind="ExternalInput")
    sc_out = nc.dram_tensor("scores", [4, BS // 4], f32, kind="ExternalOutput")

    with TileContext(nc) as tc:
        with tc.tile_pool(name="persist", bufs=1) as pp:
            # ---- constants / inputs resident in SBUF
            wt32 = pp.tile([P, HIDDEN], f32)
            for k in range(KCH):
                nc.sync.dma_start(wt32[:, k * EMB:(k + 1) * EMB],
                                  w_in.ap()[k * 128:(k + 1) * 128, :])
            wt = pp.tile([P, HIDDEN], f16)
            nc.vector.tensor_copy(wt[:], wt32[:])

            b32 = pp.tile([1, EMB], f32)
            nc.sync.dma_start(b32[:], b_in.ap())
            bh = pp.tile([1, EMB], f16)
            nc.vector.tensor_copy(bh[:], b32[:])
            ones_row = pp.tile([1, P], f16)
            nc.vector.memset(ones_row[:], 1.0)

            ident32 = pp.tile([P, P], f32)
            make_identity(nc, ident32[:])

            g1s = pp.tile([P, u_fix // 16], i16)
            nc.sync.dma_start(g1s[:], g1_in.ap())
            gqs = pp.tile([P, NQ // 16], i16)
            nc.sync.dma_start(gqs[:], gq_in.ap())
            gds = pp.tile([P, ND // 16], i16)
            nc.sync.dma_start(gds[:], gd_in.ap())
            qms = pp.tile([P, BS // 4], f32)
            nc.sync.dma_start(qms[:], qm_in.ap())
            gms = pp.tile([P, 4], f32)
            nc.sync.dma_start(gms[:], g_in.ap())

            lt = pp.tile([P, n_stripes * 128], f16)
            nc.vector.memset(lt[:, u_fix:u_fix + 128], 0.0)   # zero stripe

            if internal_table:
                # timing-only: give the un-uploaded table defined (zero) data
                ztile = pp.tile([P, HIDDEN], f32)
                nc.vector.memset(ztile[:], 0.0)
                nfull = VOCAB // P
                for zi in range(nfull):
                    nc.sync.dma_start(table.ap()[zi * P:(zi + 1) * P, :], ztile[:])
                nc.sync.dma_start(table.ap()[nfull * P:, :],
                                  ztile[:VOCAB - nfull * P, :])

            import os
            _phase = os.environ.get("K_PHASE", "AB")
            loop_cm = (tc.For_i(0, repeat, 1) if repeat is not None
                       else contextlib.nullcontext())
            with loop_cm:
                # ---- phase A: gather unique rows, project, normalize into LT
                if "A" not in _phase:
                    nc.vector.memset(lt[:, :u_fix], 0.0)
                with (
                    tc.tile_pool(name="phA", bufs=4) as pa,
                    tc.tile_pool(name="psT", bufs=3, space="PSUM") as psT,
                    tc.tile_pool(name="psA", bufs=2, space="PSUM") as psA,
                ):
                    tile_starts = (list(range(0, n_tiles, CH))
                                   if ("A" in _phase or "G" in _phase) else [])
                    for c0 in tile_starts:
                        nt = min(CH, n_tiles - c0)
                        hraw = pa.tile([P, nt, HIDDEN], f32, tag="hraw")
                        nc.gpsimd.dma_gather(
                            hraw[:], table.ap(), g1s[:, c0 * 8:c0 * 8 + nt * 8],
                            nt * 128, nt * 128, HIDDEN)
                        if "G" in _phase:
                            jb = pa.tile([P, 1], f32, tag="jb")
                            nc.vector.reduce_max(jb[:], hraw[:, 0, :128],
                                                 axis=mybir.AxisListType.X)
                            continue
                        for j in range(nt):
                            t = c0 + j
                            pt = psT.tile([P, HIDDEN], f32, tag="pt")
                            for k in range(KCH):
                                nc.tensor.transpose(pt[:, k * 128:(k + 1) * 128],
                                                    hraw[:, j, k * 128:(k + 1) * 128],
                                                    ident32[:])
                            ht = pa.tile([P, HIDDEN], f16, tag="ht")
                            nc.vector.tensor_copy(ht[:, :HIDDEN // 2], pt[:, :HIDDEN // 2])
                            nc.scalar.copy(ht[:, HIDDEN // 2:], pt[:, HIDDEN // 2:])
                            pj = psA.tile([P, EMB], f32, tag="pj")
                            for k in range(KCH):
                                nc.tensor.matmul(pj[:], lhsT=ht[:, k * 128:(k + 1) * 128],
                                                 rhs=wt[:, k * 128:(k + 1) * 128],
                                                 start=(k == 0), stop=False)
                            nc.tensor.matmul(pj[:], lhsT=ones_row[:], rhs=bh[:],
                                             start=False, stop=True)
                            sq = pa.tile([P, EMB], f16, tag="sq")
                            nsq = pa.tile([P, 1], f32, tag="nsq")
                            nc.scalar.activation(sq[:], pj[:],
                                                 mybir.ActivationFunctionType.Square,
                                                 accum_out=nsq[:])
                            nrm = pa.tile([P, 1], f32, tag="nrm")
                            nc.scalar.activation(nrm[:], nsq[:],
                                                 mybir.ActivationFunctionType.Sqrt)
                            inv = pa.tile([P, 1], f32, tag="inv")
                            nc.vector.reciprocal(inv[:], nrm[:])
                            nc.vector.tensor_scalar_mul(lt[:, t * 128:(t + 1) * 128],
                                                        pj[:], inv[:])

                # ---- phase B: token gathers + per-sample maxsim
                with (
                    tc.tile_pool(name="phB", bufs=1) as pb,
                    tc.tile_pool(name="psB", bufs=3, space="PSUM") as psB,
                ):
                  if "H" in _phase:
                    qT = pb.tile([P, 1, NQ], f16)
                    nc.gpsimd.dma_gather(qT[:], lt[:], gqs[:], NQ, NQ, EMB,
                                         transpose=True, sbuf_tokens_per_rank=128,
                                         sbuf_free_dim_per_rank=256,
                                         single_packet=False, queue_num=1)
                    dT = pb.tile([P, 1, ND], f16)
                    for p0 in range(0, ND, 4096):
                        pe = min(4096, ND - p0)
                        nc.gpsimd.dma_gather(dT[:, :, p0:p0 + pe], lt[:],
                                             gds[:, p0 // 16:(p0 + pe) // 16],
                                             pe, pe, EMB,
                                             transpose=True, sbuf_tokens_per_rank=128,
                                             sbuf_free_dim_per_rank=256,
                                             single_packet=False,
                                             queue_num=(p0 // 4096) % 2)
                    jb2 = pb.tile([P, 2], f16)
                    nc.vector.tensor_copy(jb2[:, 0:1], qT[:, 0, 0:1])
                    nc.vector.tensor_copy(jb2[:, 1:2], dT[:, 0, 0:1])
                    fino = pb.tile([4, BS // 4], f32)
                    nc.vector.memset(fino[:], 0.0)
                    nc.sync.dma_start(sc_out.ap(), fino[:])
                  elif "B" not in _phase:
                    fino = pb.tile([4, BS // 4], f32)
                    nc.vector.memset(fino[:], 0.0)
                    nc.sync.dma_start(sc_out.ap(), fino[:])
                  else:
                      qT = pb.tile([P, 1, NQ], f16)
                      nc.gpsimd.dma_gather(qT[:], lt[:], gqs[:], NQ, NQ, EMB,
                                           transpose=True, sbuf_tokens_per_rank=128,
                                           sbuf_free_dim_per_rank=256,
                                           single_packet=False)
                      dT = pb.tile([P, 1, ND], f16)
                      for p0 in range(0, ND, 4096):
                          pe = min(4096, ND - p0)
                          nc.gpsimd.dma_gather(dT[:, :, p0:p0 + pe], lt[:],
                                               gds[:, p0 // 16:(p0 + pe) // 16],
                                               pe, pe, EMB,
                                               transpose=True, sbuf_tokens_per_rank=128,
                                               sbuf_free_dim_per_rank=256,
                                               single_packet=False)

                      maxb = pb.tile([P, BS // 4], f32)
                      for g in range(BS // 4):
                          simp = psB.tile([P, 512], f32, tag="simp")
                          for j in range(4):
                              s = 4 * g + j
                              nc.tensor.matmul(simp[32 * j:32 * (j + 1), :DLEN],
                                               lhsT=qT[:, 0, QLEN * s:QLEN * (s + 1)],
                                               rhs=dT[:, 0, DLEN * s:DLEN * (s + 1)],
                                               start=True, stop=True,
                                               tile_position=(0, 32 * j))
                          nc.vector.reduce_max(maxb[:, g:g + 1], simp[:, :DLEN],
                                               axis=mybir.AxisListType.X)

                      wb = pb.tile([P, BS // 4], f32)
                      nc.vector.tensor_mul(wb[:], maxb[:], qms[:])
                      finp = psB.tile([4, BS // 4], f32, tag="finp")
                      nc.tensor.matmul(finp[:], lhsT=gms[:], rhs=wb[:],
                                       start=True, stop=True)
                      fino = pb.tile([4, BS // 4], f32)
                      nc.vector.tensor_copy(fino[:], finp[:])
                      nc.sync.dma_start(sc_out.ap(), fino[:])

    nc.compile()
    return nc


def _host_prep(query_input_ids, query_attention_mask, doc_input_ids,
               doc_attention_mask, embed_table, proj_w, proj_b):
    """Build per-core input maps + the fixed unique-count. Index-only work."""
    per_core = []
    u_counts = []
    for c in range(N_CORES):
        sl = slice(c * BS, (c + 1) * BS)
        uniq, slot_q, slot_d = _prep_core(
            np.asarray(query_input_ids[sl]), np.asarray(query_attention_mask[sl]),
            np.asarray(doc_input_ids[sl]), np.asarray(doc_attention_mask[sl]))
        per_core.append((uniq, slot_q, slot_d))
        u_counts.append(len(uniq))
    u_fix = max(128, -(-max(u_counts) // 128) * 128)

    gmat = np.zeros((128, 4), np.float32)
    for j in range(4):
        gmat[32 * j:32 * (j + 1), j] = 1.0

    table_f32 = np.ascontiguousarray(np.asarray(embed_table, dtype=np.float32))
    w_f32 = np.ascontiguousarray(np.asarray(proj_w, dtype=np.float32))
    b_f32 = np.ascontiguousarray(np.asarray(proj_b, dtype=np.float32).reshape(1, EMB))

    in_maps = []
    for c in range(N_CORES):
        uniq, slot_q, slot_d = per_core[c]
        sl = slice(c * BS, (c + 1) * BS)
        g1 = np.zeros(u_fix, np.int64)
        g1[:len(uniq)] = uniq
        dm = np.asarray(doc_attention_mask[sl]).reshape(-1)
        slot_d = slot_d.copy()
        masked = dm == 0
        slot_d[masked] = u_fix + (np.nonzero(masked)[0] % 128)
        qm = np.asarray(query_attention_mask[sl]).astype(np.float32)  # (BS, QLEN)
        # qmask_r[32*(s%4)+q, s//4] = qm[s, q]
        qmask_r = np.ascontiguousarray(
            qm.reshape(BS // 4, 4, QLEN).transpose(1, 2, 0).reshape(128, BS // 4))
        in_maps.append({
            "table": table_f32,
            "w": w_f32,
            "b": b_f32,
            "g1_idx": _wrap_idx(g1),
            "gq_idx": _wrap_idx(slot_q),
            "gd_idx": _wrap_idx(slot_d),
            "qmask_r": qmask_r,
            "gmat": gmat,
        })
    return u_fix, in_maps


def kernel(query_input_ids, query_attention_mask, doc_input_ids,
           doc_attention_mask, embed_table, proj_w, proj_b):
    from concourse.bass_utils import run_bass_kernel_spmd

    u_fix, in_maps = _host_prep(
        query_input_ids, query_attention_mask, doc_input_ids,
        doc_attention_mask, embed_table, proj_w, proj_b)

    nc = _PROG_CACHE.get(u_fix)
    if nc is None:
        nc = _build_program(u_fix)
        _PROG_CACHE[u_fix] = nc

    res = run_bass_kernel_spmd(nc, in_maps, core_ids=list(range(N_CORES)))

    scores = np.empty(B, np.float32)
    for c in range(N_CORES):
        out = res.results[c]["scores"]          # [4, BS//4]
        scores[c * BS:(c + 1) * BS] = out.T.reshape(-1)
    return scores

